# revision 30
# baseline (speedup 1.0000x reference)
"""Trainium2 Bass kernel for nn_CAM (channel attention module).

Reference computation (per batch element n):
    v = x[n].reshape(C, H*W)                      # [512, 4096]
    energy = v @ v.T                              # [512, 512]
    attn = softmax(energy, axis=-1)
    out = attn @ v                                # [512, 4096]
    result = para_mu * out + x[n]

Sharding: data-parallel over batch N=8, one batch element per NeuronCore
(8 cores). Everything is core-local — no collectives.

Kernel strategy (per core). HW calibration showed the baseline was
DMA-bound (pure-DMA variant measured 41.3us of the 55.5us total, with the
fp8-twin SBUF->SBUF cast-DMA costing ~11us of that) while the cost model
showed DVE ~97% busy; this version cuts both:
  1. Input lands as bf16 via SWDGE cast-DMA (f32 HBM read, bf16 SBUF
     write), one DMA per channel row tile. The fp8 twin V8 (output-matmul
     rhs) is now built by COMPUTE-engine copies (split ACT/DVE), not DMA:
     HBM/fabric DMA per rep drops to loads (8 MB read) + stores (4 MB
     write) only.
  2. vT column slabs via TensorE transposes in bf16 (1 cycle/row); the 8
     transposes of one k-pair land in one PSUM bank and move to SBUF with
     ONE copy that casts to the fp8 DoubleRow layout. The 16 drains are
     split Pool/ACT/DVE (they were the #1 DVE consumer); energy runs
     fp8e4 DoubleRow (contraction 256/matmul) m-outer with E[0] pipelined
     against the transpose stream.
  3. PSUM is statically banked: T0/T1 transpose ping-pong, E0/E1 energy
     ping-pong (each E[m] drains to SBUF right after its stop, freeing
     the bank), O0-O3 output accumulators. expT tiles reuse the E banks.
     This removes the cross-body serialization of the baseline's 6-bank
     accumulator rotation (body i+1's energy waited on body i's LAST
     output chunk).
  4. Row softmax per row tile m: reduce_max on Pool (negated), Exp on ACT
     writing bf16 with f32 accumulated row sum, reciprocal on DVE. The
     pm/rowsum scale is folded INTO the exp rows (one 4x-mode DVE
     tensor_scalar_mul per row) so the phase-4 epilogue needs no
     per-partition scale.
  5. Output matmul fp8e4 DoubleRow (0.5 cycles/row) into f32 PSUM (TRN2
     requires f32 matmul PSUM). Epilogue per 512-wide chunk is either
     ACT copy PSUM->bf16 + DVE bf16 add (2x) of the residual, or a single
     DVE add reading PSUM directly — chunk pattern chosen to balance
     ACT/DVE. Results stage into a [128, 4096] bf16 tile shipped as
     512/256/256 KB DMAs on the SP HWDGE queue.
  6. Queue discipline for the in-order engine queues: Pool gets only
     head/mid work (SWDGE load triggers, drains, reduces) so the next
     body's load triggers are never stuck behind tail work; ACT/DVE tail
     work (epilogue) is followed only by next-body work with a full body
     of slack (V8 casts are consumed one phase later).
  7. The benchmark hardware loop is UNROLL(=16)x-unrolled with V/V8
     double-buffered; tc.For_i places an all-engine barrier at each
     iteration boundary, so cross-rep prefetch only happens inside the
     unrolled body.
"""

import sys

if "/opt/trn_rl_repo" not in sys.path:
    sys.path.insert(0, "/opt/trn_rl_repo")

from contextlib import ExitStack

import numpy as np

import concourse.bass as bass
import concourse.mybir as mybir
import concourse.tile as tile
from concourse import bacc
from concourse.bass_utils import run_bass_kernel_spmd
from concourse.masks import make_identity

N, C, H, W = 8, 512, 64, 64
HW = H * W            # 4096
P = 128               # partitions
MT = C // P           # 4 row tiles of the channel dim
KT = HW // P          # 32 contraction tiles for the energy matmul
KP = KT // 2          # 16 k-pairs (fp8 DoubleRow contraction 256)
NCH = 512             # free-dim chunk for the output matmul (one PSUM bank)
NCHUNKS = HW // NCH   # 8
UNROLL = 16           # bodies per For_i iteration (amortizes the barrier)
F32 = mybir.dt.float32
BF16 = mybir.dt.bfloat16
F8 = mybir.dt.float8e4
DR = mybir.MatmulPerfMode.DoubleRow

# --- engine assignment (tunable) ---------------------------------------
# NOTE: GPSIMD (Pool) cannot access PSUM on TRN2 — PSUM drains are ACT/DVE
# only. V8 goes on the gpsimd cast-DMA queue (the DMA pool has slack in
# the phase-1 window; compute-engine casts measured worse).
# vts8 PSUM->SBUF drains, by k-pair: A=ACT, V=DVE. kp=0 leads the DVE
# queue so the first energy matmul is never gated on ACT's queue head.
DRAIN_ENG = "VAAAVAAAVAAAVAAA"
assert len(DRAIN_ENG) == KP
# expT PSUM->SBUF fp8 drains, by row tile mi
EXPT_ENG = "AVAV"
# epilogue per chunk: A = ACT copy PSUM->bf16 + DVE bf16 residual add
# (2x); V = single DVE add reading f32 PSUM (1x). Scale is pre-folded
# into exp_s, so no per-partition scale is needed here.
EPI_ENG = "AVAAVAAV" * 4
assert len(EPI_ENG) == MT * NCHUNKS


def _body(ctx: ExitStack, tc: "tile.TileContext", out: bass.AP, x: bass.AP, pm: bass.AP,
          reps: int = 1, mode: str = "full"):
    nc = tc.nc
    consts = ctx.enter_context(tc.tile_pool(name="consts", bufs=1))
    v_pool = ctx.enter_context(tc.tile_pool(name="v", bufs=2))
    v8_pool = ctx.enter_context(tc.tile_pool(name="v8", bufs=2))
    vt_pool = ctx.enter_context(tc.tile_pool(name="vt", bufs=1))
    exp_pool = ctx.enter_context(tc.tile_pool(name="exp", bufs=1))
    expt_pool = ctx.enter_context(tc.tile_pool(name="expt", bufs=1))
    stat_pool = ctx.enter_context(tc.tile_pool(name="stats", bufs=1))
    out_pool = ctx.enter_context(tc.tile_pool(name="ob", bufs=2))
    e_psum = ctx.enter_context(tc.tile_pool(name="e_ps", bufs=1, space="PSUM"))
    t_psum = ctx.enter_context(tc.tile_pool(name="t_ps", bufs=2, space="PSUM"))
    o_psum = ctx.enter_context(tc.tile_pool(name="o_ps", bufs=4, space="PSUM"))

    identity = consts.tile([P, P], F32)
    nc.vector.memset(identity, 0.0)
    make_identity(nc, identity, nomemset=True)
    # bf16 twin for transpose-mode matmuls of bf16 data (1 cycle/row).
    identity_bf = consts.tile([P, P], BF16)
    nc.vector.tensor_copy(out=identity_bf, in_=identity)

    # emitted after make_identity: the gpsimd queue is serial, and this DMA
    # ahead of affine_select would delay the first transposes
    pm_tile = consts.tile([P, 1], F32)
    nc.gpsimd.dma_start(out=pm_tile, in_=pm.to_broadcast((P, 1)))

    pools = (consts, v_pool, v8_pool, vt_pool, exp_pool, expt_pool, stat_pool,
             out_pool, e_psum, t_psum, o_psum)
    if reps > 1:
        # Benchmark mode: execute the body `reps` times in one NEFF via a
        # hardware loop so per-rep time is measurable over dispatch overhead.
        unroll = UNROLL if reps % UNROLL == 0 else 2
        assert reps % unroll == 0, reps
        with tc.For_i(0, reps // unroll, 1,
                      hint_engines=(mybir.EngineType.PE,
                                    mybir.EngineType.DVE,
                                    mybir.EngineType.Activation)):
            for _ in range(unroll):
                _phases(tc, out, x, pm_tile, identity, identity_bf, *pools,
                        mode=mode)
    else:
        _phases(tc, out, x, pm_tile, identity, identity_bf, *pools, mode=mode)


def _eng(tc, code: str):
    nc = tc.nc
    return {"A": nc.scalar, "V": nc.vector, "P": nc.gpsimd}[code]


def _copy(tc, code: str, out, in_):
    # ACT has no tensor_copy; scalar.copy is the activation-based copy.
    if code == "A":
        return tc.nc.scalar.copy(out, in_)
    return _eng(tc, code).tensor_copy(out=out, in_=in_)


def _phases(tc, out, x, pm_tile, identity, identity_bf,
            consts, v_pool, v8_pool, vt_pool, exp_pool, expt_pool, stat_pool,
            out_pool, e_psum, t_psum, o_psum, mode: str = "full"):
    nc = tc.nc
    # Load v as bf16 in natural layout: one [128, 4, 4096] tile ([p, m, w],
    # channel row-tile m on the free axis). SWDGE cast-DMA (only gpsimd can
    # cast); 16 KB/partition contiguous reads.
    # ONE cast-DMA for the whole input: the kp=0 transposes need all four
    # row tiles anyway, so finer DMA granularity buys no pipelining and
    # costs 3 extra SWDGE setups.
    V = v_pool.tile([P, MT, HW], BF16, name="v", tag="v")
    xv = x.rearrange("(m p) w -> p m w", p=P)
    nc.gpsimd.dma_start(out=V, in_=xv)

    # fp8 twin of v for the DoubleRow output matmul — SBUF->SBUF cast-DMAs
    # on the gpsimd SWDGE queue (compute-engine casts measured worse: the
    # DMA pool has slack during phase 1-3, the engines don't).
    V8 = v8_pool.tile([P, MT, HW], F8, name="v8", tag="v8")
    nc.gpsimd.dma_start(out=V8, in_=V)

    if mode == "dma":
        # diagnostic: HBM load + store + V8 traffic, no compute
        for mi in range(MT):
            nc.sync.dma_start(out=out[mi * P:(mi + 1) * P, :], in_=V[:, mi, :])
        return

    # Phase 1: per k-pair kp, transpose the two [512, 128] column slabs of v
    # (8 transposes) into one [128, 8, 128] bf16 PSUM tile (T0/T1
    # ping-pong), then ONE copy to SBUF casting to the fp8 DoubleRow layout
    # vts8[p, kp, ko, c] = vT[kp*256 + ko*128 + p, c]. Drains are split
    # Pool/ACT/DVE by DRAIN_ENG. Energy is m-outer and symmetric (row tile
    # m computes column blocks j >= [0,1,2,2][m]); E[0]'s matmuls are
    # software-pipelined one k-pair behind the transpose stream so row 0's
    # softmax overlaps rows 1-3's energy matmuls.
    SYM_LO = [0, 1, 2, 2]
    vts8 = vt_pool.tile([P, KP, 2, C], F8, name="vts8", tag="vts8")
    E = [None] * MT
    E[0] = e_psum.tile([P, C], F32, name="e0", tag="e0")
    for kp in range(KP + 1):
        if kp < KP:
            tp = t_psum.tile([P, 2 * MT, P], BF16, tag="tp")
            for ko in range(2):
                for m in range(MT):
                    kb = 2 * kp + ko
                    nc.tensor.transpose(
                        tp[:, ko * MT + m, :], V[:, m, kb * P:(kb + 1) * P],
                        identity_bf,
                    )
            _copy(tc, DRAIN_ENG[kp], vts8[:, kp, :, :],
                  tp.rearrange("p (ko m) q -> p ko (m q)", ko=2))
        if kp >= 1:
            kk = kp - 1
            nc.tensor.matmul(
                E[0],
                lhsT=vts8[:, kk, :, 0:P],
                rhs=vts8[:, kk, :, :],
                start=(kk == 0),
                stop=(kk == KP - 1),
                perf_mode=DR,
            )

    # Rows m>=1 energy + per-row softmax chain. Each E[m] drains to SBUF
    # right after its stop (E banks ping-pong E0/E1: E[2] waits only on
    # E[0]'s early drain). Missing lower blocks j < SYM_LO[m] are
    # reconstructed as transposes of earlier rows' computed blocks.
    RECON = {0: [], 1: [(1, 0)], 2: [(2, 0), (2, 1)], 3: [(3, 0), (3, 1)]}
    E_sb = [None] * MT
    EXP_S = [None] * MT

    def softmax_row(mi):
        esb = exp_pool.tile([P, C], F32, name=f"esb{mi}", tag=f"esb{mi}")
        lo = SYM_LO[mi] * P
        nc.vector.tensor_copy(out=esb[:, lo:], in_=E[mi][:, lo:])
        E_sb[mi] = esb
        for ti, tj in RECON[mi]:
            tp = t_psum.tile([P, MT, P], F32, tag="tp")
            nc.tensor.transpose(tp[:, 0, :], E_sb[tj][:, ti * P:(ti + 1) * P],
                                identity)
            nc.vector.tensor_copy(out=esb[:, tj * P:(tj + 1) * P], in_=tp[:, 0, :])
        neg_max = stat_pool.tile([P, 1], F32, tag=f"negm{mi}")
        nc.vector.tensor_reduce(
            out=neg_max,
            in_=esb,
            op=mybir.AluOpType.max,
            axis=mybir.AxisListType.X,
            negate=True,
        )
        exp_t = exp_pool.tile([P, C], BF16, name=f"exp{mi}", tag=f"exp{mi}")
        s_t = stat_pool.tile([P, 1], F32, tag=f"s{mi}")
        nc.scalar.activation(
            out=exp_t,
            in_=esb,
            func=mybir.ActivationFunctionType.Exp,
            bias=neg_max,
            scale=1.0,
            accum_out=s_t,
        )
        # fold pm/rowsum into the exp rows: exp_s = exp_t * (1/rowsum) * pm
        # with both scalar multiplies fused into one tensor_scalar op
        # (divide is not a valid tensor_scalar ALU op on the DVE ISA).
        rs = stat_pool.tile([P, 1], F32, tag=f"rs{mi}")
        nc.vector.reciprocal(rs, s_t)
        exp_s = exp_pool.tile([P, C], BF16, name=f"expS{mi}", tag=f"expS{mi}")
        nc.vector.tensor_scalar(
            out=exp_s, in0=exp_t, scalar1=rs, scalar2=pm_tile,
            op0=mybir.AluOpType.mult, op1=mybir.AluOpType.mult,
        )
        EXP_S[mi] = exp_s

    softmax_row(0)
    for m in range(1, MT):
        lo = SYM_LO[m] * P
        E[m] = e_psum.tile([P, C], F32, name=f"e{m}", tag=f"e{m % 2}")
        for kp in range(KP):
            nc.tensor.matmul(
                E[m][:, lo:],
                lhsT=vts8[:, kp, :, m * P:(m + 1) * P],
                rhs=vts8[:, kp, :, lo:],
                start=(kp == 0),
                stop=(kp == KP - 1),
                perf_mode=DR,
            )
        softmax_row(m)

    if mode == "phase1":
        # diagnostic: everything up to softmax, plus the output stores
        for mi in range(MT):
            nc.sync.dma_start(out=out[mi * P:(mi + 1) * P, :], in_=V[:, mi, :])
        return

    # Phase 4: out rows = expT_scaled.T @ v in fp8e4 DoubleRow. expT tiles
    # go through the E banks (free after the energy drains; body i+1's
    # E[0] then waits only on this body's expT drains, which happen early
    # in phase 4 — not on the last output chunk). Output accumulators
    # rotate over the 4 O banks. Epilogue per chunk: residual add only
    # (scale was folded into exp_s), engine per EPI_ENG.
    EXPT = expt_pool.tile([P, MT, C], F8, name="expt", tag="expt")

    def emit_expt_block(mi):
        # transpose in bf16 (fp8 PSUM outputs fail the BIR verifier); the
        # PSUM->SBUF copy does the fp8 cast.
        tp = e_psum.tile([P, MT, P], BF16, tag=f"e{mi % 2}", name=f"tpx{mi}")
        for mj in range(MT):
            nc.tensor.transpose(tp[:, mj, :], EXP_S[mi][:, mj * P:(mj + 1) * P],
                                identity_bf)
        _copy(tc, EXPT_ENG[mi], EXPT[:, :, mi * P:(mi + 1) * P], tp)

    emit_expt_block(0)
    for mi in range(MT):
        if mi + 1 < MT:
            emit_expt_block(mi + 1)  # one row ahead: copy overlaps mi's matmuls
        ob = None if mode == "noepi" else out_pool.tile([P, HW], BF16, tag="ob")
        for cidx in range(NCHUNKS):
            o_ps = o_psum.tile([P, NCH], F32, name=f"ops{mi}_{cidx}", tag="ops")
            vslice = V[:, mi, cidx * NCH:(cidx + 1) * NCH]
            for j in range(MT // 2):
                nc.tensor.matmul(
                    o_ps,
                    lhsT=EXPT[:, 2 * j:2 * j + 2, mi * P:(mi + 1) * P],
                    rhs=V8[:, 2 * j:2 * j + 2, cidx * NCH:(cidx + 1) * NCH],
                    start=(j == 0),
                    stop=(j == MT // 2 - 1),
                    perf_mode=DR,
                )
            if mode != "noepi":
                obc = ob[:, cidx * NCH:(cidx + 1) * NCH]
                if EPI_ENG[mi * NCHUNKS + cidx] == "A":
                    nc.scalar.copy(obc, o_ps)
                    nc.vector.tensor_add(obc, obc, vslice)
                else:
                    nc.vector.tensor_add(obc, o_ps, vslice)
            src = V[:, mi, :] if mode == "noepi" else ob
            if cidx == NCHUNKS // 2 - 1:
                nc.sync.dma_start(
                    out=out[mi * P:(mi + 1) * P, :HW // 2], in_=src[:, :HW // 2]
                )
            elif cidx == NCHUNKS - 3:
                nc.sync.dma_start(
                    out=out[mi * P:(mi + 1) * P, HW // 2:HW * 3 // 4],
                    in_=src[:, HW // 2:HW * 3 // 4],
                )
        src = V[:, mi, :] if mode == "noepi" else ob
        nc.sync.dma_start(
            out=out[mi * P:(mi + 1) * P, HW * 3 // 4:], in_=src[:, HW * 3 // 4:]
        )


def build_nc(reps: int = 1, mode: str = "full") -> bass.Bass:
    # bacc.Bacc (not raw bass.Bass): its compile() pass legalizes multi-sem
    # waits into explicit event-semaphore instructions (walrus allows only one
    # sync wait per TPB instruction).
    nc = bacc.Bacc("TRN2", debug=False)
    x = nc.dram_tensor("x", [C, HW], F32, kind="ExternalInput").ap()
    pm = nc.dram_tensor("para_mu", [1], F32, kind="ExternalInput").ap()
    out = nc.dram_tensor("out", [C, HW], BF16, kind="ExternalOutput").ap()
    with tile.TileContext(nc) as tc, ExitStack() as ctx:
        _body(ctx, tc, out, x, pm, reps=reps, mode=mode)
    nc.compile()
    return nc


_nc_cache = None


def run(x: np.ndarray, para_mu: np.ndarray, **spmd_kwargs):
    """Run on 8 NeuronCores; returns (output [8,512,64,64], BassKernelResults)."""
    global _nc_cache
    x = np.ascontiguousarray(np.asarray(x, dtype=np.float32))
    pm = np.ascontiguousarray(np.asarray(para_mu, dtype=np.float32).reshape(1))
    assert x.shape == (N, C, H, W), x.shape
    if _nc_cache is None:
        _nc_cache = build_nc()
    in_maps = [
        {"x": x[n].reshape(C, HW), "para_mu": pm} for n in range(N)
    ]
    res = run_bass_kernel_spmd(_nc_cache, in_maps, core_ids=list(range(N)), **spmd_kwargs)
    out = np.stack(
        [np.asarray(res.results[n]["out"]).astype(np.float32).reshape(C, H, W)
         for n in range(N)]
    )
    return out, res


def kernel(x: np.ndarray, para_mu: np.ndarray) -> np.ndarray:
    out, _ = run(x, para_mu)
    return out


# revision 32
# speedup vs baseline: 1.0424x; 1.0424x over previous
"""Trainium2 Bass kernel for nn_CAM (channel attention module).

Reference computation (per batch element n):
    v = x[n].reshape(C, H*W)                      # [512, 4096]
    energy = v @ v.T                              # [512, 512]
    attn = softmax(energy, axis=-1)
    out = attn @ v                                # [512, 4096]
    result = para_mu * out + x[n]

Sharding: data-parallel over batch N=8, one batch element per NeuronCore
(8 cores). Everything is core-local — no collectives.

Kernel strategy (per core). HW calibration showed the baseline was
DMA-bound (pure-DMA variant measured 41.3us of the 55.5us total, with the
fp8-twin SBUF->SBUF cast-DMA costing ~11us of that) while the cost model
showed DVE ~97% busy; this version cuts both:
  1. Input lands as bf16 via SWDGE cast-DMA (f32 HBM read, bf16 SBUF
     write), one DMA per channel row tile. The fp8 twin V8 (output-matmul
     rhs) is now built by COMPUTE-engine copies (split ACT/DVE), not DMA:
     HBM/fabric DMA per rep drops to loads (8 MB read) + stores (4 MB
     write) only.
  2. vT column slabs via TensorE transposes in bf16 (1 cycle/row); the 8
     transposes of one k-pair land in one PSUM bank and move to SBUF with
     ONE copy that casts to the fp8 DoubleRow layout. The 16 drains are
     split Pool/ACT/DVE (they were the #1 DVE consumer); energy runs
     fp8e4 DoubleRow (contraction 256/matmul) m-outer with E[0] pipelined
     against the transpose stream.
  3. PSUM is statically banked: T0/T1 transpose ping-pong, E0/E1 energy
     ping-pong (each E[m] drains to SBUF right after its stop, freeing
     the bank), O0-O3 output accumulators. expT tiles reuse the E banks.
     This removes the cross-body serialization of the baseline's 6-bank
     accumulator rotation (body i+1's energy waited on body i's LAST
     output chunk).
  4. Row softmax per row tile m: reduce_max on Pool (negated), Exp on ACT
     writing bf16 with f32 accumulated row sum, reciprocal on DVE. The
     pm/rowsum scale is folded INTO the exp rows (one 4x-mode DVE
     tensor_scalar_mul per row) so the phase-4 epilogue needs no
     per-partition scale.
  5. Output matmul fp8e4 DoubleRow (0.5 cycles/row) into f32 PSUM (TRN2
     requires f32 matmul PSUM). Epilogue per 512-wide chunk is either
     ACT copy PSUM->bf16 + DVE bf16 add (2x) of the residual, or a single
     DVE add reading PSUM directly — chunk pattern chosen to balance
     ACT/DVE. Results stage into a [128, 4096] bf16 tile shipped as
     512/256/256 KB DMAs on the SP HWDGE queue.
  6. Queue discipline for the in-order engine queues: Pool gets only
     head/mid work (SWDGE load triggers, drains, reduces) so the next
     body's load triggers are never stuck behind tail work; ACT/DVE tail
     work (epilogue) is followed only by next-body work with a full body
     of slack (V8 casts are consumed one phase later).
  7. The benchmark hardware loop is UNROLL(=16)x-unrolled with V/V8
     double-buffered; tc.For_i places an all-engine barrier at each
     iteration boundary, so cross-rep prefetch only happens inside the
     unrolled body.
"""

import sys

if "/opt/trn_rl_repo" not in sys.path:
    sys.path.insert(0, "/opt/trn_rl_repo")

from contextlib import ExitStack

import numpy as np

import concourse.bass as bass
import concourse.mybir as mybir
import concourse.tile as tile
from concourse import bacc
from concourse.bass_utils import run_bass_kernel_spmd
from concourse.masks import make_identity

N, C, H, W = 8, 512, 64, 64
HW = H * W            # 4096
P = 128               # partitions
MT = C // P           # 4 row tiles of the channel dim
KT = HW // P          # 32 contraction tiles for the energy matmul
KP = KT // 2          # 16 k-pairs (fp8 DoubleRow contraction 256)
NCH = 512             # free-dim chunk for the output matmul (one PSUM bank)
NCHUNKS = HW // NCH   # 8
UNROLL = 16           # bodies per For_i iteration (amortizes the barrier)
F32 = mybir.dt.float32
BF16 = mybir.dt.bfloat16
F8 = mybir.dt.float8e4
DR = mybir.MatmulPerfMode.DoubleRow

# --- engine assignment (tunable) ---------------------------------------
# NOTE: GPSIMD (Pool) cannot access PSUM on TRN2 — PSUM drains are ACT/DVE
# only. V8 goes on the gpsimd cast-DMA queue (the DMA pool has slack in
# the phase-1 window; compute-engine casts measured worse).
# vts8 PSUM->SBUF drains, by k-pair: A=ACT, V=DVE. kp=0 leads the DVE
# queue so the first energy matmul is never gated on ACT's queue head.
DRAIN_ENG = "VAAAVAAAVAAAVAAA"
assert len(DRAIN_ENG) == KP
# expT PSUM->SBUF fp8 drains, by row tile mi
EXPT_ENG = "AVAV"
# epilogue per chunk: A = ACT copy PSUM->bf16 + DVE bf16 residual add
# (2x); V = single DVE add reading f32 PSUM (1x). Scale is pre-folded
# into exp_s, so no per-partition scale is needed here.
EPI_ENG = "AVAAVAAV" * 4
assert len(EPI_ENG) == MT * NCHUNKS


def _body(ctx: ExitStack, tc: "tile.TileContext", out: bass.AP, x: bass.AP, pm: bass.AP,
          reps: int = 1, mode: str = "full"):
    nc = tc.nc
    consts = ctx.enter_context(tc.tile_pool(name="consts", bufs=1))
    v_pool = ctx.enter_context(tc.tile_pool(name="v", bufs=2))
    v8_pool = ctx.enter_context(tc.tile_pool(name="v8", bufs=2))
    vt_pool = ctx.enter_context(tc.tile_pool(name="vt", bufs=1))
    exp_pool = ctx.enter_context(tc.tile_pool(name="exp", bufs=1))
    expt_pool = ctx.enter_context(tc.tile_pool(name="expt", bufs=1))
    stat_pool = ctx.enter_context(tc.tile_pool(name="stats", bufs=1))
    out_pool = ctx.enter_context(tc.tile_pool(name="ob", bufs=2))
    e_psum = ctx.enter_context(tc.tile_pool(name="e_ps", bufs=1, space="PSUM"))
    t_psum = ctx.enter_context(tc.tile_pool(name="t_ps", bufs=2, space="PSUM"))
    o_psum = ctx.enter_context(tc.tile_pool(name="o_ps", bufs=4, space="PSUM"))

    identity = consts.tile([P, P], F32)
    nc.vector.memset(identity, 0.0)
    make_identity(nc, identity, nomemset=True)
    # bf16 twin for transpose-mode matmuls of bf16 data (1 cycle/row).
    identity_bf = consts.tile([P, P], BF16)
    nc.vector.tensor_copy(out=identity_bf, in_=identity)

    # emitted after make_identity: the gpsimd queue is serial, and this DMA
    # ahead of affine_select would delay the first transposes
    pm_tile = consts.tile([P, 1], F32)
    nc.gpsimd.dma_start(out=pm_tile, in_=pm.to_broadcast((P, 1)))

    pools = (consts, v_pool, v8_pool, vt_pool, exp_pool, expt_pool, stat_pool,
             out_pool, e_psum, t_psum, o_psum)
    if reps > 1:
        # Benchmark mode: execute the body `reps` times in one NEFF via a
        # hardware loop so per-rep time is measurable over dispatch overhead.
        unroll = UNROLL if reps % UNROLL == 0 else 2
        assert reps % unroll == 0, reps
        with tc.For_i(0, reps // unroll, 1,
                      hint_engines=(mybir.EngineType.PE,
                                    mybir.EngineType.DVE,
                                    mybir.EngineType.Activation)):
            for _ in range(unroll):
                _phases(tc, out, x, pm_tile, identity, identity_bf, *pools,
                        mode=mode)
    else:
        _phases(tc, out, x, pm_tile, identity, identity_bf, *pools, mode=mode)


def _eng(tc, code: str):
    nc = tc.nc
    return {"A": nc.scalar, "V": nc.vector, "P": nc.gpsimd}[code]


def _copy(tc, code: str, out, in_):
    # ACT has no tensor_copy; scalar.copy is the activation-based copy.
    if code == "A":
        return tc.nc.scalar.copy(out, in_)
    return _eng(tc, code).tensor_copy(out=out, in_=in_)


def _phases(tc, out, x, pm_tile, identity, identity_bf,
            consts, v_pool, v8_pool, vt_pool, exp_pool, expt_pool, stat_pool,
            out_pool, e_psum, t_psum, o_psum, mode: str = "full"):
    nc = tc.nc
    # Load v as bf16 in natural layout: one [128, 4, 4096] tile ([p, m, w],
    # channel row-tile m on the free axis). SWDGE cast-DMA (only gpsimd can
    # cast); 16 KB/partition contiguous reads.
    V = v_pool.tile([P, MT, HW], BF16, name="v", tag="v")
    xv = x.rearrange("(m p) w -> p m w", p=P)
    for m in range(MT):
        nc.gpsimd.dma_start(out=V[:, m, :], in_=xv[:, m, :])

    # fp8 twin of v for the DoubleRow output matmul — SBUF->SBUF cast-DMAs
    # on the gpsimd SWDGE queue (compute-engine casts measured worse: the
    # DMA pool has slack during phase 1-3, the engines don't).
    V8 = v8_pool.tile([P, MT, HW], F8, name="v8", tag="v8")
    for m in range(MT):
        nc.gpsimd.dma_start(out=V8[:, m, :], in_=V[:, m, :])

    if mode == "dma":
        # diagnostic: HBM load + store + V8 traffic, no compute
        for mi in range(MT):
            nc.sync.dma_start(out=out[mi * P:(mi + 1) * P, :], in_=V[:, mi, :])
        return

    # Phase 1: per k-pair kp, transpose the two [512, 128] column slabs of v
    # (8 transposes) into one [128, 8, 128] bf16 PSUM tile (T0/T1
    # ping-pong), then ONE copy to SBUF casting to the fp8 DoubleRow layout
    # vts8[p, kp, ko, c] = vT[kp*256 + ko*128 + p, c]. Drains are split
    # Pool/ACT/DVE by DRAIN_ENG. Energy is m-outer and symmetric (row tile
    # m computes column blocks j >= [0,1,2,2][m]); E[0]'s matmuls are
    # software-pipelined one k-pair behind the transpose stream so row 0's
    # softmax overlaps rows 1-3's energy matmuls.
    SYM_LO = [0, 1, 2, 2]
    vts8 = vt_pool.tile([P, KP, 2, C], F8, name="vts8", tag="vts8")
    E = [None] * MT
    E[0] = e_psum.tile([P, C], F32, name="e0", tag="e0")
    for kp in range(KP + 1):
        if kp < KP:
            tp = t_psum.tile([P, 2 * MT, P], BF16, tag="tp")
            for ko in range(2):
                for m in range(MT):
                    kb = 2 * kp + ko
                    nc.tensor.transpose(
                        tp[:, ko * MT + m, :], V[:, m, kb * P:(kb + 1) * P],
                        identity_bf,
                    )
            _copy(tc, DRAIN_ENG[kp], vts8[:, kp, :, :],
                  tp.rearrange("p (ko m) q -> p ko (m q)", ko=2))
        if kp >= 1:
            kk = kp - 1
            nc.tensor.matmul(
                E[0],
                lhsT=vts8[:, kk, :, 0:P],
                rhs=vts8[:, kk, :, :],
                start=(kk == 0),
                stop=(kk == KP - 1),
                perf_mode=DR,
            )

    # Rows m>=1 energy + per-row softmax chain. Each E[m] drains to SBUF
    # right after its stop (E banks ping-pong E0/E1: E[2] waits only on
    # E[0]'s early drain). Missing lower blocks j < SYM_LO[m] are
    # reconstructed as transposes of earlier rows' computed blocks.
    RECON = {0: [], 1: [(1, 0)], 2: [(2, 0), (2, 1)], 3: [(3, 0), (3, 1)]}
    E_sb = [None] * MT
    EXP_S = [None] * MT

    def softmax_row(mi):
        esb = exp_pool.tile([P, C], F32, name=f"esb{mi}", tag=f"esb{mi}")
        lo = SYM_LO[mi] * P
        nc.vector.tensor_copy(out=esb[:, lo:], in_=E[mi][:, lo:])
        E_sb[mi] = esb
        for ti, tj in RECON[mi]:
            tp = t_psum.tile([P, MT, P], F32, tag="tp")
            nc.tensor.transpose(tp[:, 0, :], E_sb[tj][:, ti * P:(ti + 1) * P],
                                identity)
            nc.vector.tensor_copy(out=esb[:, tj * P:(tj + 1) * P], in_=tp[:, 0, :])
        neg_max = stat_pool.tile([P, 1], F32, tag=f"negm{mi}")
        nc.vector.tensor_reduce(
            out=neg_max,
            in_=esb,
            op=mybir.AluOpType.max,
            axis=mybir.AxisListType.X,
            negate=True,
        )
        exp_t = exp_pool.tile([P, C], BF16, name=f"exp{mi}", tag=f"exp{mi}")
        s_t = stat_pool.tile([P, 1], F32, tag=f"s{mi}")
        nc.scalar.activation(
            out=exp_t,
            in_=esb,
            func=mybir.ActivationFunctionType.Exp,
            bias=neg_max,
            scale=1.0,
            accum_out=s_t,
        )
        # fold pm/rowsum into the exp rows: exp_s = exp_t * (1/rowsum) * pm
        # with both scalar multiplies fused into one tensor_scalar op
        # (divide is not a valid tensor_scalar ALU op on the DVE ISA).
        rs = stat_pool.tile([P, 1], F32, tag=f"rs{mi}")
        nc.vector.reciprocal(rs, s_t)
        exp_s = exp_pool.tile([P, C], BF16, name=f"expS{mi}", tag=f"expS{mi}")
        nc.vector.tensor_scalar(
            out=exp_s, in0=exp_t, scalar1=rs, scalar2=pm_tile,
            op0=mybir.AluOpType.mult, op1=mybir.AluOpType.mult,
        )
        EXP_S[mi] = exp_s

    softmax_row(0)
    for m in range(1, MT):
        lo = SYM_LO[m] * P
        E[m] = e_psum.tile([P, C], F32, name=f"e{m}", tag=f"e{m % 2}")
        for kp in range(KP):
            nc.tensor.matmul(
                E[m][:, lo:],
                lhsT=vts8[:, kp, :, m * P:(m + 1) * P],
                rhs=vts8[:, kp, :, lo:],
                start=(kp == 0),
                stop=(kp == KP - 1),
                perf_mode=DR,
            )
        softmax_row(m)

    if mode == "phase1":
        # diagnostic: everything up to softmax, plus the output stores
        for mi in range(MT):
            nc.sync.dma_start(out=out[mi * P:(mi + 1) * P, :], in_=V[:, mi, :])
        return

    # Phase 4: out rows = expT_scaled.T @ v in fp8e4 DoubleRow. expT tiles
    # go through the E banks (free after the energy drains; body i+1's
    # E[0] then waits only on this body's expT drains, which happen early
    # in phase 4 — not on the last output chunk). Output accumulators
    # rotate over the 4 O banks. Epilogue per chunk: residual add only
    # (scale was folded into exp_s), engine per EPI_ENG.
    EXPT = expt_pool.tile([P, MT, C], F8, name="expt", tag="expt")

    def emit_expt_block(mi):
        # transpose in bf16 (fp8 PSUM outputs fail the BIR verifier); the
        # PSUM->SBUF copy does the fp8 cast.
        tp = e_psum.tile([P, MT, P], BF16, tag=f"e{mi % 2}", name=f"tpx{mi}")
        for mj in range(MT):
            nc.tensor.transpose(tp[:, mj, :], EXP_S[mi][:, mj * P:(mj + 1) * P],
                                identity_bf)
        _copy(tc, EXPT_ENG[mi], EXPT[:, :, mi * P:(mi + 1) * P], tp)

    emit_expt_block(0)
    for mi in range(MT):
        if mi + 1 < MT:
            emit_expt_block(mi + 1)  # one row ahead: copy overlaps mi's matmuls
        ob = None if mode == "noepi" else out_pool.tile([P, HW], BF16, tag="ob")
        for cidx in range(NCHUNKS):
            o_ps = o_psum.tile([P, NCH], F32, name=f"ops{mi}_{cidx}", tag="ops")
            vslice = V[:, mi, cidx * NCH:(cidx + 1) * NCH]
            for j in range(MT // 2):
                nc.tensor.matmul(
                    o_ps,
                    lhsT=EXPT[:, 2 * j:2 * j + 2, mi * P:(mi + 1) * P],
                    rhs=V8[:, 2 * j:2 * j + 2, cidx * NCH:(cidx + 1) * NCH],
                    start=(j == 0),
                    stop=(j == MT // 2 - 1),
                    perf_mode=DR,
                )
            if mode != "noepi":
                obc = ob[:, cidx * NCH:(cidx + 1) * NCH]
                if EPI_ENG[mi * NCHUNKS + cidx] == "A":
                    nc.scalar.copy(obc, o_ps)
                    nc.vector.tensor_add(obc, obc, vslice)
                else:
                    nc.vector.tensor_add(obc, o_ps, vslice)
            src = V[:, mi, :] if mode == "noepi" else ob
            if cidx == NCHUNKS // 2 - 1:
                nc.sync.dma_start(
                    out=out[mi * P:(mi + 1) * P, :HW // 2], in_=src[:, :HW // 2]
                )
            elif cidx == NCHUNKS - 3:
                nc.sync.dma_start(
                    out=out[mi * P:(mi + 1) * P, HW // 2:HW * 3 // 4],
                    in_=src[:, HW // 2:HW * 3 // 4],
                )
        src = V[:, mi, :] if mode == "noepi" else ob
        nc.sync.dma_start(
            out=out[mi * P:(mi + 1) * P, HW * 3 // 4:], in_=src[:, HW * 3 // 4:]
        )


def build_nc(reps: int = 1, mode: str = "full") -> bass.Bass:
    # bacc.Bacc (not raw bass.Bass): its compile() pass legalizes multi-sem
    # waits into explicit event-semaphore instructions (walrus allows only one
    # sync wait per TPB instruction).
    nc = bacc.Bacc("TRN2", debug=False)
    x = nc.dram_tensor("x", [C, HW], F32, kind="ExternalInput").ap()
    pm = nc.dram_tensor("para_mu", [1], F32, kind="ExternalInput").ap()
    out = nc.dram_tensor("out", [C, HW], BF16, kind="ExternalOutput").ap()
    with tile.TileContext(nc) as tc, ExitStack() as ctx:
        _body(ctx, tc, out, x, pm, reps=reps, mode=mode)
    nc.compile()
    return nc


_nc_cache = None


def run(x: np.ndarray, para_mu: np.ndarray, **spmd_kwargs):
    """Run on 8 NeuronCores; returns (output [8,512,64,64], BassKernelResults)."""
    global _nc_cache
    x = np.ascontiguousarray(np.asarray(x, dtype=np.float32))
    pm = np.ascontiguousarray(np.asarray(para_mu, dtype=np.float32).reshape(1))
    assert x.shape == (N, C, H, W), x.shape
    if _nc_cache is None:
        _nc_cache = build_nc()
    in_maps = [
        {"x": x[n].reshape(C, HW), "para_mu": pm} for n in range(N)
    ]
    res = run_bass_kernel_spmd(_nc_cache, in_maps, core_ids=list(range(N)), **spmd_kwargs)
    out = np.stack(
        [np.asarray(res.results[n]["out"]).astype(np.float32).reshape(C, H, W)
         for n in range(N)]
    )
    return out, res


def kernel(x: np.ndarray, para_mu: np.ndarray) -> np.ndarray:
    out, _ = run(x, para_mu)
    return out


# revision 39
# speedup vs baseline: 1.0650x; 1.0217x over previous
"""Trainium2 Bass kernel for nn_CAM (channel attention module).

Reference computation (per batch element n):
    v = x[n].reshape(C, H*W)                      # [512, 4096]
    energy = v @ v.T                              # [512, 512]
    attn = softmax(energy, axis=-1)
    out = attn @ v                                # [512, 4096]
    result = para_mu * out + x[n]

Sharding: data-parallel over batch N=8, one batch element per NeuronCore
(8 cores). Everything is core-local — no collectives.

Kernel strategy (per core). HW calibration: the pure-DMA variant (loads +
fp8 twin + stores, no compute) measures ~41us, so the body is largely
DMA-floor-bound; the cost model put DVE ~97% busy in the 56.6us baseline.
This version rebalances the vector engines and removes the baseline's
cross-body PSUM serialization (HW: 56.6us -> ~49us):
  1. Input lands as bf16 via SWDGE cast-DMA (f32 HBM read, bf16 SBUF
     write), one DMA per channel row tile; the fp8 twin V8 (output-matmul
     rhs) follows on the same gpsimd queue (compute-engine casts for V8
     measured WORSE: the DMA pool has slack in the phase 1-3 window, the
     vector engines do not).
  2. vT column slabs via TensorE transposes in bf16 (1 cycle/row); the 8
     transposes of one k-pair land in one PSUM bank and move to SBUF with
     ONE copy that casts to the fp8 DoubleRow layout. The 16 drains are
     split ACT/DVE per DRAIN_ENG (they were the #1 DVE consumer; GPSIMD
     cannot touch PSUM on TRN2). Energy runs fp8e4 DoubleRow (contraction
     256/matmul) m-outer with E[0] pipelined against the transposes.
  3. PSUM is statically banked: T0/T1 transpose ping-pong, E0/E1 energy
     ping-pong (each E[m] drains to SBUF right after its stop, freeing
     the bank), O0-O3 output accumulators. expT transpose tiles reuse the
     E banks. This removes the baseline's cross-body serialization
     (body i+1's energy waited on body i's LAST output chunk).
  4. Row softmax per row tile m: DVE reduce_max (negated), ACT Exp
     writing bf16 with f32 accumulated row sum, DVE reciprocal, then ONE
     fused tensor_scalar (4x mode) folds pm/rowsum into the exp rows, so
     the phase-4 epilogue needs no per-partition scale.
  5. Output matmul fp8e4 DoubleRow (0.5 cycles/row) into f32 PSUM (TRN2
     matmuls cannot write 16-bit PSUM; an identity-matmul residual into
     PSUM measured ~20us WORSE — bf16<->fp8 PE mode switches break the
     DR stream). Epilogue per 512-wide chunk per EPI_ENG: either ACT copy
     PSUM->bf16 + DVE bf16 residual add (2x rate), or a single DVE add
     reading PSUM (1x). Results stage into a [128, 4096] bf16 tile
     shipped as 512/256/256 KB DMAs on the SP HWDGE queue.
  6. Queue discipline for the in-order engine queues: Pool carries only
     DMA triggers; ACT/DVE tail work (epilogue) is followed only by
     next-body work whose consumers sit a phase later, and drain kp=0
     leads the DVE queue so the first energy matmul never waits on ACT's
     queue head.
  7. The benchmark hardware loop is UNROLL(=16)x-unrolled with V/V8
     double-buffered; tc.For_i places an all-engine barrier at each
     iteration boundary, so cross-rep prefetch only happens inside the
     unrolled body.
"""

import sys

if "/opt/trn_rl_repo" not in sys.path:
    sys.path.insert(0, "/opt/trn_rl_repo")

from contextlib import ExitStack

import numpy as np

import concourse.bass as bass
import concourse.mybir as mybir
import concourse.tile as tile
from concourse import bacc
from concourse.bass_utils import run_bass_kernel_spmd
from concourse.masks import make_identity

N, C, H, W = 8, 512, 64, 64
HW = H * W            # 4096
P = 128               # partitions
MT = C // P           # 4 row tiles of the channel dim
KT = HW // P          # 32 contraction tiles for the energy matmul
KP = KT // 2          # 16 k-pairs (fp8 DoubleRow contraction 256)
NCH = 512             # free-dim chunk for the output matmul (one PSUM bank)
NCHUNKS = HW // NCH   # 8
UNROLL = 16           # bodies per For_i iteration (amortizes the barrier)
F32 = mybir.dt.float32
BF16 = mybir.dt.bfloat16
F8 = mybir.dt.float8e4
DR = mybir.MatmulPerfMode.DoubleRow

# --- engine assignment (tunable) ---------------------------------------
# NOTE: GPSIMD (Pool) cannot access PSUM on TRN2 — PSUM drains are ACT/DVE
# only. V8 goes on the gpsimd cast-DMA queue (the DMA pool has slack in
# the phase-1 window; compute-engine casts measured worse).
# vts8 PSUM->SBUF drains, by k-pair: A=ACT, V=DVE. kp=0 leads the DVE
# queue so the first energy matmul is never gated on ACT's queue head.
DRAIN_ENG = "VAVAVAVAVAVAVAAA"
assert len(DRAIN_ENG) == KP
# expT PSUM->SBUF fp8 drains, by row tile mi
EXPT_ENG = "AVAV"
# epilogue per chunk: A = ACT copy PSUM->bf16 + DVE bf16 residual add
# (2x); V = single DVE add reading f32 PSUM (1x). Scale is pre-folded
# into exp_s, so no per-partition scale is needed here.
EPI_ENG = "AVAAVAAV" * 4
assert len(EPI_ENG) == MT * NCHUNKS
# number of output-store DMAs per row tile (2, 3, or 4)
STORE_SPLIT = 4


def _body(ctx: ExitStack, tc: "tile.TileContext", out: bass.AP, x: bass.AP, pm: bass.AP,
          reps: int = 1, mode: str = "full"):
    nc = tc.nc
    consts = ctx.enter_context(tc.tile_pool(name="consts", bufs=1))
    v_pool = ctx.enter_context(tc.tile_pool(name="v", bufs=2))
    v8_pool = ctx.enter_context(tc.tile_pool(name="v8", bufs=2))
    vt_pool = ctx.enter_context(tc.tile_pool(name="vt", bufs=1))
    exp_pool = ctx.enter_context(tc.tile_pool(name="exp", bufs=1))
    expt_pool = ctx.enter_context(tc.tile_pool(name="expt", bufs=1))
    stat_pool = ctx.enter_context(tc.tile_pool(name="stats", bufs=1))
    out_pool = ctx.enter_context(tc.tile_pool(name="ob", bufs=2))
    e_psum = ctx.enter_context(tc.tile_pool(name="e_ps", bufs=1, space="PSUM"))
    t_psum = ctx.enter_context(tc.tile_pool(name="t_ps", bufs=2, space="PSUM"))
    o_psum = ctx.enter_context(tc.tile_pool(name="o_ps", bufs=4, space="PSUM"))

    identity = consts.tile([P, P], F32)
    nc.vector.memset(identity, 0.0)
    make_identity(nc, identity, nomemset=True)
    # bf16 twin for transpose-mode matmuls of bf16 data (1 cycle/row).
    identity_bf = consts.tile([P, P], BF16)
    nc.vector.tensor_copy(out=identity_bf, in_=identity)

    # emitted after make_identity: the gpsimd queue is serial, and this DMA
    # ahead of affine_select would delay the first transposes
    pm_tile = consts.tile([P, 1], F32)
    nc.gpsimd.dma_start(out=pm_tile, in_=pm.to_broadcast((P, 1)))

    pools = (consts, v_pool, v8_pool, vt_pool, exp_pool, expt_pool, stat_pool,
             out_pool, e_psum, t_psum, o_psum)
    if reps > 1:
        # Benchmark mode: execute the body `reps` times in one NEFF via a
        # hardware loop so per-rep time is measurable over dispatch overhead.
        unroll = UNROLL if reps % UNROLL == 0 else 2
        assert reps % unroll == 0, reps
        with tc.For_i(0, reps // unroll, 1,
                      hint_engines=(mybir.EngineType.PE,
                                    mybir.EngineType.DVE,
                                    mybir.EngineType.Activation)):
            for _ in range(unroll):
                _phases(tc, out, x, pm_tile, identity, identity_bf, *pools,
                        mode=mode)
    else:
        _phases(tc, out, x, pm_tile, identity, identity_bf, *pools, mode=mode)


def _eng(tc, code: str):
    nc = tc.nc
    return {"A": nc.scalar, "V": nc.vector, "P": nc.gpsimd}[code]


def _copy(tc, code: str, out, in_):
    # ACT has no tensor_copy; scalar.copy is the activation-based copy.
    if code == "A":
        return tc.nc.scalar.copy(out, in_)
    return _eng(tc, code).tensor_copy(out=out, in_=in_)


def _phases(tc, out, x, pm_tile, identity, identity_bf,
            consts, v_pool, v8_pool, vt_pool, exp_pool, expt_pool, stat_pool,
            out_pool, e_psum, t_psum, o_psum, mode: str = "full"):
    nc = tc.nc
    # Load v as bf16 in natural layout: one [128, 4, 4096] tile ([p, m, w],
    # channel row-tile m on the free axis). SWDGE cast-DMA (only gpsimd can
    # cast); 16 KB/partition contiguous reads.
    V = v_pool.tile([P, MT, HW], BF16, name="v", tag="v")
    xv = x.rearrange("(m p) w -> p m w", p=P)
    for m in range(MT):
        nc.gpsimd.dma_start(out=V[:, m, :], in_=xv[:, m, :])

    # fp8 twin of v for the DoubleRow output matmul — SBUF->SBUF cast-DMAs
    # on the gpsimd SWDGE queue (compute-engine casts measured worse: the
    # DMA pool has slack during phase 1-3, the engines don't).
    V8 = v8_pool.tile([P, MT, HW], F8, name="v8", tag="v8")
    for m in range(MT):
        nc.gpsimd.dma_start(out=V8[:, m, :], in_=V[:, m, :])

    if mode == "dma":
        # diagnostic: HBM load + store + V8 traffic, no compute
        for mi in range(MT):
            nc.sync.dma_start(out=out[mi * P:(mi + 1) * P, :], in_=V[:, mi, :])
        return

    # Phase 1: per k-pair kp, transpose the two [512, 128] column slabs of v
    # (8 transposes) into one [128, 8, 128] bf16 PSUM tile (T0/T1
    # ping-pong), then ONE copy to SBUF casting to the fp8 DoubleRow layout
    # vts8[p, kp, ko, c] = vT[kp*256 + ko*128 + p, c]. Drains are split
    # ACT/DVE by DRAIN_ENG. Energy is m-outer and symmetric (row tile
    # m computes column blocks j >= [0,1,2,2][m]); E[0]'s matmuls are
    # software-pipelined one k-pair behind the transpose stream so row 0's
    # softmax overlaps rows 1-3's energy matmuls.
    SYM_LO = [0, 1, 2, 2]
    vts8 = vt_pool.tile([P, KP, 2, C], F8, name="vts8", tag="vts8")
    E = [None] * MT
    E[0] = e_psum.tile([P, C], F32, name="e0", tag="e0")
    for kp in range(KP + 1):
        if kp < KP:
            tp = t_psum.tile([P, 2 * MT, P], BF16, tag="tp")
            for ko in range(2):
                for m in range(MT):
                    kb = 2 * kp + ko
                    nc.tensor.transpose(
                        tp[:, ko * MT + m, :], V[:, m, kb * P:(kb + 1) * P],
                        identity_bf,
                    )
            _copy(tc, DRAIN_ENG[kp], vts8[:, kp, :, :],
                  tp.rearrange("p (ko m) q -> p ko (m q)", ko=2))
        if kp >= 1:
            kk = kp - 1
            nc.tensor.matmul(
                E[0],
                lhsT=vts8[:, kk, :, 0:P],
                rhs=vts8[:, kk, :, :],
                start=(kk == 0),
                stop=(kk == KP - 1),
                perf_mode=DR,
            )

    # Rows m>=1 energy + per-row softmax chain. Each E[m] drains to SBUF
    # right after its stop (E banks ping-pong E0/E1: E[2] waits only on
    # E[0]'s early drain). Missing lower blocks j < SYM_LO[m] are
    # reconstructed as transposes of earlier rows' computed blocks.
    RECON = {0: [], 1: [(1, 0)], 2: [(2, 0), (2, 1)], 3: [(3, 0), (3, 1)]}
    E_sb = [None] * MT
    EXP_S = [None] * MT

    def softmax_row(mi):
        esb = exp_pool.tile([P, C], F32, name=f"esb{mi}", tag=f"esb{mi}")
        lo = SYM_LO[mi] * P
        nc.vector.tensor_copy(out=esb[:, lo:], in_=E[mi][:, lo:])
        E_sb[mi] = esb
        for ti, tj in RECON[mi]:
            tp = t_psum.tile([P, MT, P], F32, tag="tp")
            nc.tensor.transpose(tp[:, 0, :], E_sb[tj][:, ti * P:(ti + 1) * P],
                                identity)
            nc.vector.tensor_copy(out=esb[:, tj * P:(tj + 1) * P], in_=tp[:, 0, :])
        neg_max = stat_pool.tile([P, 1], F32, tag=f"negm{mi}")
        nc.vector.tensor_reduce(
            out=neg_max,
            in_=esb,
            op=mybir.AluOpType.max,
            axis=mybir.AxisListType.X,
            negate=True,
        )
        exp_t = exp_pool.tile([P, C], BF16, name=f"exp{mi}", tag=f"exp{mi}")
        s_t = stat_pool.tile([P, 1], F32, tag=f"s{mi}")
        nc.scalar.activation(
            out=exp_t,
            in_=esb,
            func=mybir.ActivationFunctionType.Exp,
            bias=neg_max,
            scale=1.0,
            accum_out=s_t,
        )
        # fold pm/rowsum into the exp rows: exp_s = exp_t * (1/rowsum) * pm
        # with both scalar multiplies fused into one tensor_scalar op
        # (divide is not a valid tensor_scalar ALU op on the DVE ISA).
        rs = stat_pool.tile([P, 1], F32, tag=f"rs{mi}")
        nc.vector.reciprocal(rs, s_t)
        exp_s = exp_pool.tile([P, C], BF16, name=f"expS{mi}", tag=f"expS{mi}")
        nc.vector.tensor_scalar(
            out=exp_s, in0=exp_t, scalar1=rs, scalar2=pm_tile,
            op0=mybir.AluOpType.mult, op1=mybir.AluOpType.mult,
        )
        EXP_S[mi] = exp_s

    softmax_row(0)
    for m in range(1, MT):
        lo = SYM_LO[m] * P
        E[m] = e_psum.tile([P, C], F32, name=f"e{m}", tag=f"e{m % 2}")
        for kp in range(KP):
            nc.tensor.matmul(
                E[m][:, lo:],
                lhsT=vts8[:, kp, :, m * P:(m + 1) * P],
                rhs=vts8[:, kp, :, lo:],
                start=(kp == 0),
                stop=(kp == KP - 1),
                perf_mode=DR,
            )
        softmax_row(m)

    if mode == "phase1":
        # diagnostic: everything up to softmax, plus the output stores
        for mi in range(MT):
            nc.sync.dma_start(out=out[mi * P:(mi + 1) * P, :], in_=V[:, mi, :])
        return

    # Phase 4: out rows = expT_scaled.T @ v in fp8e4 DoubleRow. expT tiles
    # go through the E banks (free after the energy drains; body i+1's
    # E[0] then waits only on this body's expT drains, which happen early
    # in phase 4 — not on the last output chunk). Output accumulators
    # rotate over the 4 O banks. Epilogue per chunk: residual add only
    # (scale was folded into exp_s), engine per EPI_ENG.
    EXPT = expt_pool.tile([P, MT, C], F8, name="expt", tag="expt")

    def emit_expt_block(mi):
        # transpose in bf16 (fp8 PSUM outputs fail the BIR verifier); the
        # PSUM->SBUF copy does the fp8 cast.
        tp = e_psum.tile([P, MT, P], BF16, tag=f"e{mi % 2}", name=f"tpx{mi}")
        for mj in range(MT):
            nc.tensor.transpose(tp[:, mj, :], EXP_S[mi][:, mj * P:(mj + 1) * P],
                                identity_bf)
        _copy(tc, EXPT_ENG[mi], EXPT[:, :, mi * P:(mi + 1) * P], tp)

    emit_expt_block(0)
    for mi in range(MT):
        if mi + 1 < MT:
            emit_expt_block(mi + 1)  # one row ahead: copy overlaps mi's matmuls
        ob = None if mode == "noepi" else out_pool.tile([P, HW], BF16, tag="ob")
        for cidx in range(NCHUNKS):
            o_ps = o_psum.tile([P, NCH], F32, name=f"ops{mi}_{cidx}", tag="ops")
            vslice = V[:, mi, cidx * NCH:(cidx + 1) * NCH]
            for j in range(MT // 2):
                nc.tensor.matmul(
                    o_ps,
                    lhsT=EXPT[:, 2 * j:2 * j + 2, mi * P:(mi + 1) * P],
                    rhs=V8[:, 2 * j:2 * j + 2, cidx * NCH:(cidx + 1) * NCH],
                    start=(j == 0),
                    stop=(j == MT // 2 - 1),
                    perf_mode=DR,
                )
            if mode != "noepi":
                obc = ob[:, cidx * NCH:(cidx + 1) * NCH]
                if EPI_ENG[mi * NCHUNKS + cidx] == "A":
                    nc.scalar.copy(obc, o_ps)
                    nc.vector.tensor_add(obc, obc, vslice)
                else:
                    nc.vector.tensor_add(obc, o_ps, vslice)
            src = V[:, mi, :] if mode == "noepi" else ob
            # progressive stores: ship completed chunks early so the store
            # tail after the last chunk stays short.
            if STORE_SPLIT >= 8:
                nc.sync.dma_start(
                    out=out[mi * P:(mi + 1) * P, cidx * NCH:(cidx + 1) * NCH],
                    in_=src[:, cidx * NCH:(cidx + 1) * NCH],
                )
            elif STORE_SPLIT >= 4 and cidx == NCHUNKS // 4 - 1:
                nc.sync.dma_start(
                    out=out[mi * P:(mi + 1) * P, :HW // 4], in_=src[:, :HW // 4]
                )
            elif STORE_SPLIT < 8 and cidx == NCHUNKS // 2 - 1:
                lo = HW // 4 if STORE_SPLIT >= 4 else 0
                nc.sync.dma_start(
                    out=out[mi * P:(mi + 1) * P, lo:HW // 2],
                    in_=src[:, lo:HW // 2],
                )
            elif STORE_SPLIT >= 3 and STORE_SPLIT < 8 and cidx == NCHUNKS - 3:
                nc.sync.dma_start(
                    out=out[mi * P:(mi + 1) * P, HW // 2:HW * 3 // 4],
                    in_=src[:, HW // 2:HW * 3 // 4],
                )
        if STORE_SPLIT < 8:
            src = V[:, mi, :] if mode == "noepi" else ob
            lo = HW * 3 // 4 if STORE_SPLIT >= 3 else HW // 2
            nc.sync.dma_start(
                out=out[mi * P:(mi + 1) * P, lo:], in_=src[:, lo:]
            )


def build_nc(reps: int = 1, mode: str = "full") -> bass.Bass:
    # bacc.Bacc (not raw bass.Bass): its compile() pass legalizes multi-sem
    # waits into explicit event-semaphore instructions (walrus allows only one
    # sync wait per TPB instruction).
    nc = bacc.Bacc("TRN2", debug=False)
    x = nc.dram_tensor("x", [C, HW], F32, kind="ExternalInput").ap()
    pm = nc.dram_tensor("para_mu", [1], F32, kind="ExternalInput").ap()
    out = nc.dram_tensor("out", [C, HW], BF16, kind="ExternalOutput").ap()
    with tile.TileContext(nc) as tc, ExitStack() as ctx:
        _body(ctx, tc, out, x, pm, reps=reps, mode=mode)
    nc.compile()
    return nc


_nc_cache = None


def run(x: np.ndarray, para_mu: np.ndarray, **spmd_kwargs):
    """Run on 8 NeuronCores; returns (output [8,512,64,64], BassKernelResults)."""
    global _nc_cache
    x = np.ascontiguousarray(np.asarray(x, dtype=np.float32))
    pm = np.ascontiguousarray(np.asarray(para_mu, dtype=np.float32).reshape(1))
    assert x.shape == (N, C, H, W), x.shape
    if _nc_cache is None:
        _nc_cache = build_nc()
    in_maps = [
        {"x": x[n].reshape(C, HW), "para_mu": pm} for n in range(N)
    ]
    res = run_bass_kernel_spmd(_nc_cache, in_maps, core_ids=list(range(N)), **spmd_kwargs)
    out = np.stack(
        [np.asarray(res.results[n]["out"]).astype(np.float32).reshape(C, H, W)
         for n in range(N)]
    )
    return out, res


def kernel(x: np.ndarray, para_mu: np.ndarray) -> np.ndarray:
    out, _ = run(x, para_mu)
    return out
